# revision 1
# baseline (speedup 1.0000x reference)
"""DGCNN block (knn -> edge-conv -> BN/ReLU -> conv -> BN/ReLU) on 8 trn2
NeuronCores, data-parallel over the batch (one sample per core).

v2 restructuring vs v1:
  - slab (-0.5|x_m|^2) is pre-baked into PSUM by the Activation engine;
    the distance matmuls accumulate on top (start=False), so the DVE/Pool
    tensor adds and extra copies of v1 disappear.  DVE does only
    Max/MaxIndex (the per-row top-8 scan) plus the small h/BN work.
  - gathers are 1024-wide gpsimd indirect_copies (ISA dst limit), issued
    per 8-tile column quarter as soon as that quarter's indices are
    shuffled, overlapping the remaining KNN scan.
  - the index wrap (row-major [128,8] -> wrapped [16,512] per quarter)
    is one small DMA to DRAM + one 2-byte-granular DMA back + 8 cheap
    contiguous replications, instead of v1's 64 element-granular DMAs.
"""
import sys

sys.path.insert(0, "/opt/trn_rl_repo")

import numpy as np

B, C, N = 8, 128, 4096
NT = N // 128          # 32 row tiles
NCHUNK = N // 512      # 8 column chunks
EPS = 1e-5
NEGBIG = -30000.0

_CACHE = {}


# --------------------------------------------------------------------------
# workarounds for this walrus build (small sem-wait encodings)
# --------------------------------------------------------------------------

def _patched_drain_and_barrier(self, tick_clock, wait_clock):
    from concourse.vector_clock import ScopedClock, VectorClock

    nc = self.nc
    gc = tick_clock.global_clock
    n = len(gc)
    for p in range(n):
        t = gc[p]
        if t > 0:
            vc = VectorClock([0] * n)
            vc.require_at_least(p, t)
            w = nc.sync.nop()
            wait_clock.add_sem_waits(w.ins, ScopedClock({None: vc}))
    nc.sync.drain()
    nc.all_engine_barrier()
    assert self.sems is not None
    popped = nc._tile_sem_poison_stack.pop()
    assert popped is self._sem_poison
    nc.clear_and_free_semaphores(list(self.sems.allocated().values()))
    nc.all_engine_barrier()


def _split_excess_waits(nc, cap=1):
    import concourse.mybir as mybir

    for f in nc.m.functions:
        for bb in f.blocks:
            il = bb.instructions
            k = 0
            while k < len(il):
                inst = il[k]
                si = inst.sync_info
                if si is None or not si.on_wait or len(si.on_wait) <= cap:
                    k += 1
                    continue
                waits = list(si.on_wait)
                keep, excess = waits[-cap:], waits[:-cap]
                pos = k
                for i0 in range(0, len(excess), cap):
                    chunk = excess[i0:i0 + cap]
                    nop = mybir.InstNoOp(
                        name=f"{inst.name}-wsplit{i0}", ins=[], outs=[]
                    )
                    nop.engine = inst.engine
                    nop.sync_info = mybir.SyncInfo(on_wait=chunk, on_update=[])
                    il.insert(pos, nop)
                    pos += 1
                    k += 1
                inst.sync_info = mybir.SyncInfo(
                    on_wait=keep, on_update=list(si.on_update or [])
                )
                k += 1


# --------------------------------------------------------------------------
# device program
# --------------------------------------------------------------------------

def build(collectives=True, prebake=True):
    import concourse.bass as bass
    import concourse.tile as tile
    import concourse.mybir as mybir
    from concourse.library_overlay import lower_extended_insts

    tile.TileContext._drain_and_barrier = _patched_drain_and_barrier

    f32 = mybir.dt.float32
    f16 = mybir.dt.float16
    u16 = mybir.dt.uint16

    nc = bass.Bass()

    x_d = nc.dram_tensor("x", [C, N], f32, kind="ExternalInput")
    wbase_d = nc.dram_tensor("wbaseT", [C, C], f16, kind="ExternalInput")
    negw1b_d = nc.dram_tensor("negw1bT", [C, 3 * C], f16, kind="ExternalInput")
    w2t_d = nc.dram_tensor("w2T", [C, 3 * C], f16, kind="ExternalInput")
    id16_d = nc.dram_tensor("id16", [C, C], f16, kind="ExternalInput")
    negbig_d = nc.dram_tensor("negbigI", [C, C], f16, kind="ExternalInput")
    nhm_d = nc.dram_tensor("neghalf_mat", [C, C], f32, kind="ExternalInput")
    gb_d = nc.dram_tensor("gb", [C, 4], f32, kind="ExternalInput")  # g1,beta1,g2,beta2

    out_d = nc.dram_tensor("out", [C, N], f32, kind="ExternalOutput")

    with tile.TileContext(nc) as tc:
        with (
            tc.tile_pool(name="persist", bufs=1) as pp,
            tc.tile_pool(name="work", bufs=1) as wp,
            tc.tile_pool(name="small", bufs=1) as sp,
            tc.tile_pool(name="psum", bufs=2, space="PSUM") as psp,
            tc.tile_pool(name="dram", bufs=1, space="DRAM") as dp,
        ):
            # ---------- load ----------
            x32 = wp.tile([C, N], f32, tag="big32", bufs=3)
            nc.sync.dma_start(x32[:, :N // 2], x_d[:, :N // 2])
            nc.sync.dma_start(x32[:, N // 2:], x_d[:, N // 2:])
            wbase = pp.tile([C, C], f16)
            nc.sync.dma_start(wbase[:], wbase_d[:])
            negw1b = pp.tile([C, 3 * C], f16)
            nc.sync.dma_start(negw1b[:], negw1b_d[:])
            w2t = pp.tile([C, 3 * C], f16)
            nc.sync.dma_start(w2t[:], w2t_d[:])
            id16 = pp.tile([C, C], f16)
            nc.sync.dma_start(id16[:], id16_d[:])
            negbig = pp.tile([C, C], f16)
            nc.sync.dma_start(negbig[:], negbig_d[:])
            nhm = pp.tile([C, C], f32)
            nc.sync.dma_start(nhm[:], nhm_d[:])
            nhm16 = pp.tile([C, C], f16)
            nc.scalar.copy(nhm16[:], nhm[:])
            gb = pp.tile([C, 4], f32)
            nc.sync.dma_start(gb[:], gb_d[:])

            # ---------- prep: hi/lo split, sq, slab ----------
            xhi = pp.tile([C, N], f16)
            xhi32 = wp.tile([C, N], f32, tag="big32", bufs=3)
            xlo = pp.tile([C, N], f16)
            xsq = wp.tile([C, N], f32, tag="big32", bufs=3)
            xsqhi = pp.tile([C, N], f16)
            xsqhi32 = wp.tile([C, N], f32, tag="big32", bufs=3)
            xsqlo = pp.tile([C, N], f16)
            # per-half so tile 0's early quarters start before prep finishes
            for ph2 in range(2):
                hs = slice(ph2 * (N // 2), (ph2 + 1) * (N // 2))
                nc.scalar.copy(xhi[:, hs], x32[:, hs])
                nc.scalar.copy(xhi32[:, hs], xhi[:, hs])
                nc.vector.tensor_sub(xlo[:, hs], x32[:, hs], xhi32[:, hs])
                nc.vector.tensor_mul(xsq[:, hs], x32[:, hs], x32[:, hs])
                nc.scalar.copy(xsqhi[:, hs], xsq[:, hs])
                nc.scalar.copy(xsqhi32[:, hs], xsqhi[:, hs])
                nc.vector.tensor_sub(xsqlo[:, hs], xsq[:, hs], xsqhi32[:, hs])

            # ---------- negY/base tiles (emitted inside the loop, after tile 2) ----
            negY = pp.tile([C, 3 * N], f16)   # t-major: [:, t*N + n]
            base16 = pp.tile([C, N], f16)

            def emit_negy_t(t):
                for ck in range(NCHUNK):
                    ps = psp.tile([C, 512], f32, tag="ph0", bufs=2)
                    nc.tensor.matmul(
                        ps[:], negw1b[:, t * C:(t + 1) * C],
                        xhi[:, ck * 512:(ck + 1) * 512], start=True, stop=True,
                    )
                    nc.scalar.copy(
                        negY[:, t * N + ck * 512:t * N + (ck + 1) * 512], ps[:]
                    )

            def emit_base():
                for ck in range(NCHUNK):
                    ps = psp.tile([C, 512], f32, tag="ph0", bufs=2)
                    nc.tensor.matmul(
                        ps[:], wbase[:], xhi[:, ck * 512:(ck + 1) * 512],
                        start=True, stop=True,
                    )
                    nc.scalar.copy(base16[:, ck * 512:(ck + 1) * 512], ps[:])

            # ---------- persistent state for the pipelined phase ----------
            idxall = pp.tile([C, NT * 8], u16)   # [p, r*8+k] idx of rank k+2
            iw = pp.tile([C, 4 * 512], u16)      # wrapped idx, per quarter:
            #                                      [:, qa*512 + (kk-1)*64 + rr*8 + q]
            h = [pp.tile([C, N], f16, name=f"h{j}", tag=f"h{j}") for j in range(3)]
            nstat = 3 * NCHUNK
            stats = sp.tile([C, nstat * 6], f32, tag="stats")

            def emit_topk_tile(r):
                hi_t = xhi[:, r * 128:(r + 1) * 128]
                lo_t = xlo[:, r * 128:(r + 1) * 128]
                d = wp.tile([C, N], f32, tag="dtile", bufs=2)
                ckd = r // 4                       # chunk containing diagonal
                off = 128 * (r % 4)
                for qt in range(4):
                    ph = psp.tile([C, 1024], f32, tag="ph", bufs=3)
                    qs = slice(qt * 1024, (qt + 1) * 1024)
                    pe_slab = True
                    for c2 in range(2):
                        ck = qt * 2 + c2
                        sl = ph[:, c2 * 512:(c2 + 1) * 512]
                        rs = slice(ck * 512, (ck + 1) * 512)
                        if pe_slab:
                            nc.tensor.matmul(sl, nhm16[:], xsqhi[:, rs],
                                             start=True, stop=False)
                            nc.tensor.matmul(sl, nhm16[:], xsqlo[:, rs],
                                             start=False, stop=False)
                        nc.tensor.matmul(sl, hi_t, xhi[:, rs],
                                         start=False, stop=False)
                        nc.tensor.matmul(sl, hi_t, xlo[:, rs], start=False, stop=False)
                        if ck == ckd:
                            nc.tensor.matmul(sl, lo_t, xhi[:, rs], start=False, stop=False)
                            nc.tensor.matmul(
                                sl[:, off:off + 128], id16[:], negbig[:],
                                start=False, stop=True,
                            )
                        else:
                            nc.tensor.matmul(sl, lo_t, xhi[:, rs], start=False, stop=True)
                    nc.scalar.copy(d[:, qs], ph[:])
                v8 = sp.tile([C, 8], f32, tag="v8", bufs=2)
                nc.vector.max(v8[:], d[:])
                nc.vector.max_index(idxall[:, r * 8:(r + 1) * 8], v8[:], d[:])

            def emit_quarter_shuffle(qa):
                # idxall[:, qa*64:(qa+1)*64] -> wrapped iw[:, qa*512:(qa+1)*512]
                idxdram = dp.tile([8 * 128, 8], u16, name=f"idxd{qa}")
                nc.sync.dma_start(
                    idxdram[:].rearrange("(rr p) k -> p rr k", p=128),
                    idxall[:, qa * 64:(qa + 1) * 64].rearrange(
                        "c (rr k) -> c rr k", k=8
                    ),
                )
                iwq = sp.tile([16, 512], u16, tag=f"iwq{qa}")
                src = idxdram[:].rearrange(
                    "(rr q w) k -> w (rr q) k", rr=8, q=8, w=16
                )
                for k in range(8):
                    nc.sync.dma_start(
                        iwq[:, k * 64:(k + 1) * 64],
                        src[:, :, k:k + 1].rearrange("w f a -> w (f a)"),
                    )
                for g in range(8):
                    nc.sync.dma_start(
                        iw[g * 16:(g + 1) * 16, qa * 512:(qa + 1) * 512], iwq[:]
                    )

            gq = {}

            def emit_quarter_gathers(qa):
                gt = {}
                for kk in range(1, 9):
                    g = wp.tile([C, 1024], f16, tag=f"g{kk}", bufs=1)
                    t = kk % 3
                    nc.gpsimd.indirect_copy(
                        g[:], negY[:, t * N:(t + 1) * N],
                        iw[:, qa * 512 + (kk - 1) * 64:qa * 512 + kk * 64],
                        i_know_ap_gather_is_preferred=True,
                    )
                    gt[kk] = g
                gq[qa] = gt

            def emit_quarter_assemble(qa):
                qs = slice(qa * 1024, (qa + 1) * 1024)
                gt = gq[qa]
                # h_j quarter = base + g_{3j} + g_{3j+1} + g_{3j+2} (kk=0: self)
                eng = nc.vector
                tmp = wp.tile([C, 1024], f16, tag="htmp", bufs=2)
                for j in range(3):
                    if j == 0:
                        eng.tensor_add(
                            tmp[:], base16[:, qs], negY[:, 0 * N + qa * 1024:
                                                        0 * N + (qa + 1) * 1024]
                        )
                        eng.tensor_add(tmp[:], tmp[:], gt[1][:])
                        eng.tensor_add(h[0][:, qs], tmp[:], gt[2][:])
                    else:
                        eng.tensor_add(tmp[:], base16[:, qs], gt[3 * j][:])
                        eng.tensor_add(tmp[:], tmp[:], gt[3 * j + 1][:])
                        eng.tensor_add(h[j][:, qs], tmp[:], gt[3 * j + 2][:])
                    for c2 in range(2):
                        ck = qa * 2 + c2
                        nc.vector.bn_stats(
                            stats[:, (j * NCHUNK + ck) * 6:(j * NCHUNK + ck + 1) * 6],
                            h[j][:, ck * 512:(ck + 1) * 512],
                        )

            # ---------- pipelined KNN + gather ----------
            def emit_quarter_stats(qa):
                for j in range(3):
                    for c2 in range(2):
                        ck = qa * 2 + c2
                        nc.vector.bn_stats(
                            stats[:, (j * NCHUNK + ck) * 6:(j * NCHUNK + ck + 1) * 6],
                            h[j][:, ck * 512:(ck + 1) * 512],
                        )

            for r in range(NT):
                emit_topk_tile(r)
                if 2 <= r <= 4:
                    emit_negy_t(r - 2)
                if r == 5:
                    emit_base()
                if r % 8 == 7 and r < NT - 1:
                    qa = r // 8
                    emit_quarter_shuffle(qa)
                    emit_quarter_gathers(qa)
                if r % 8 == 1 and r >= 9:
                    emit_quarter_assemble(r // 8 - 1)
            # Q4: shuffle, gather, assemble (stats inline)
            emit_quarter_shuffle(3)
            emit_quarter_gathers(3)
            emit_quarter_assemble(3)

            # ---------- BN1 reduce ----------
            mv = sp.tile([C, 2], f32, tag="mv")
            nc.vector.bn_aggr(mv[:], stats[:].rearrange("c (s k) -> c s k", k=6))

            pay = sp.tile([C, 2], f32, tag="pay")
            nc.vector.tensor_copy(pay[:, 0:1], mv[:, 0:1])
            msq = sp.tile([C, 1], f32, tag="t1")
            nc.vector.tensor_mul(msq[:], mv[:, 0:1], mv[:, 0:1])
            nc.vector.tensor_add(pay[:, 1:2], mv[:, 1:2], msq[:])

            if collectives:
                cin = dp.tile([C, 2], f32)
                cout = dp.tile([C, 2], f32)
                nc.gpsimd.dma_start(cin[:], pay[:])
                nc.gpsimd.collective_compute(
                    "AllReduce", mybir.AluOpType.add,
                    replica_groups=[list(range(B))],
                    ins=[cin[:]], outs=[cout[:]],
                )
                red = sp.tile([C, 2], f32, tag="red")
                nc.gpsimd.dma_start(red[:], cout[:])
                scale_n = 1.0 / B
            else:
                red = pay
                scale_n = 1.0

            # sc1 = g1 * rsqrt(var_g + eps); bi1 = beta1 - mean_g * sc1
            mean_g = sp.tile([C, 1], f32, tag="t2")
            nc.vector.tensor_scalar_mul(mean_g[:], red[:, 0:1], scale_n)
            ex2 = sp.tile([C, 1], f32, tag="t3")
            nc.vector.tensor_scalar_mul(ex2[:], red[:, 1:2], scale_n)
            mg2 = sp.tile([C, 1], f32, tag="t4")
            nc.vector.tensor_mul(mg2[:], mean_g[:], mean_g[:])
            var_g = sp.tile([C, 1], f32, tag="t5")
            nc.vector.tensor_sub(var_g[:], ex2[:], mg2[:])
            veps = sp.tile([C, 1], f32, tag="t6b")
            nc.vector.tensor_scalar_add(veps[:], var_g[:], EPS)
            sd = sp.tile([C, 1], f32, tag="t6")
            nc.scalar.activation(
                sd[:], veps[:], mybir.ActivationFunctionType.Sqrt
            )
            rst = sp.tile([C, 1], f32, tag="t7")
            nc.vector.reciprocal(rst[:], sd[:])
            sc1 = sp.tile([C, 1], f32, tag="sc1")
            nc.vector.tensor_mul(sc1[:], gb[:, 0:1], rst[:])
            tmp1 = sp.tile([C, 1], f32, tag="t8")
            nc.vector.tensor_mul(tmp1[:], mean_g[:], sc1[:])
            bi1 = sp.tile([C, 1], f32, tag="bi1")
            nc.vector.tensor_sub(bi1[:], gb[:, 1:2], tmp1[:])

            # ---------- BN1 apply + conv2, pipelined per quarter ----------
            o2 = wp.tile([C, N], f32, tag="big32", bufs=3)
            stats2 = sp.tile([C, NCHUNK * 6], f32, tag="stats2")
            for qa in range(4):
                qs = slice(qa * 1024, (qa + 1) * 1024)
                for j in range(3):
                    nc.scalar.activation(
                        h[j][:, qs], h[j][:, qs],
                        mybir.ActivationFunctionType.Relu,
                        bias=bi1[:], scale=sc1[:],
                    )
                for c2 in range(2):
                    ck = qa * 2 + c2
                    ps = psp.tile([C, 512], f32, tag="ph0", bufs=2)
                    for j in range(3):
                        nc.tensor.matmul(
                            ps[:], w2t[:, j * C:(j + 1) * C],
                            h[j][:, ck * 512:(ck + 1) * 512],
                            start=(j == 0), stop=(j == 2),
                        )
                    nc.scalar.copy(o2[:, ck * 512:(ck + 1) * 512], ps[:])
                    nc.vector.bn_stats(
                        stats2[:, ck * 6:(ck + 1) * 6],
                        o2[:, ck * 512:(ck + 1) * 512],
                    )

            # ---------- BN2 ----------
            mv2 = sp.tile([C, 2], f32, tag="mv2")
            nc.vector.bn_aggr(mv2[:], stats2[:].rearrange("c (s k) -> c s k", k=6))
            pay2 = sp.tile([C, 2], f32, tag="pay2")
            nc.vector.tensor_copy(pay2[:, 0:1], mv2[:, 0:1])
            msq2 = sp.tile([C, 1], f32, tag="u1")
            nc.vector.tensor_mul(msq2[:], mv2[:, 0:1], mv2[:, 0:1])
            nc.vector.tensor_add(pay2[:, 1:2], mv2[:, 1:2], msq2[:])

            if collectives:
                cin2 = dp.tile([C, 2], f32)
                cout2 = dp.tile([C, 2], f32)
                nc.gpsimd.dma_start(cin2[:], pay2[:])
                nc.gpsimd.collective_compute(
                    "AllReduce", mybir.AluOpType.add,
                    replica_groups=[list(range(B))],
                    ins=[cin2[:]], outs=[cout2[:]],
                )
                red2 = sp.tile([C, 2], f32, tag="red2")
                nc.gpsimd.dma_start(red2[:], cout2[:])
            else:
                red2 = pay2

            mean2 = sp.tile([C, 1], f32, tag="u2")
            nc.vector.tensor_scalar_mul(mean2[:], red2[:, 0:1], scale_n)
            ex22 = sp.tile([C, 1], f32, tag="u3")
            nc.vector.tensor_scalar_mul(ex22[:], red2[:, 1:2], scale_n)
            mg22 = sp.tile([C, 1], f32, tag="u4")
            nc.vector.tensor_mul(mg22[:], mean2[:], mean2[:])
            var2 = sp.tile([C, 1], f32, tag="u5")
            nc.vector.tensor_sub(var2[:], ex22[:], mg22[:])
            veps2 = sp.tile([C, 1], f32, tag="u6b")
            nc.vector.tensor_scalar_add(veps2[:], var2[:], EPS)
            sd2 = sp.tile([C, 1], f32, tag="u6")
            nc.scalar.activation(
                sd2[:], veps2[:], mybir.ActivationFunctionType.Sqrt
            )
            rst2 = sp.tile([C, 1], f32, tag="u7")
            nc.vector.reciprocal(rst2[:], sd2[:])
            sc2 = sp.tile([C, 1], f32, tag="sc2")
            nc.vector.tensor_mul(sc2[:], gb[:, 2:3], rst2[:])
            tmp2 = sp.tile([C, 1], f32, tag="u8")
            nc.vector.tensor_mul(tmp2[:], mean2[:], sc2[:])
            bi2 = sp.tile([C, 1], f32, tag="bi2")
            nc.vector.tensor_sub(bi2[:], gb[:, 3:4], tmp2[:])

            for hh in range(4):
                hs = slice(hh * 1024, (hh + 1) * 1024)
                nc.scalar.activation(
                    o2[:, hs], o2[:, hs], mybir.ActivationFunctionType.Relu,
                    bias=bi2[:], scale=sc2[:],
                )
                nc.sync.dma_start(out_d[:, hs], o2[:, hs])

    lower_extended_insts(nc)
    _split_excess_waits(nc)
    return nc


# --------------------------------------------------------------------------
# host wrapper
# --------------------------------------------------------------------------

def _prep_shared(w1, w2, g1, beta1, g2, beta2):
    w1 = np.asarray(w1, np.float32)
    w2 = np.asarray(w2, np.float32)
    W1A, W1B = w1[:, :C, :], w1[:, C:, :]
    wbaseT = (W1A + W1B).sum(2).T.astype(np.float16).copy()
    negw1bT = np.concatenate(
        [(-W1B[:, :, t]).T for t in range(3)], axis=1
    ).astype(np.float16)
    w2T = np.concatenate([w2[:, :, j].T for j in range(3)], axis=1).astype(np.float16)
    id16 = np.eye(C, dtype=np.float16)
    negbigI = (NEGBIG * np.eye(C)).astype(np.float16)
    neghalf_mat = np.full((C, C), -0.5, np.float32)
    gb = np.stack(
        [np.asarray(g1, np.float32), np.asarray(beta1, np.float32),
         np.asarray(g2, np.float32), np.asarray(beta2, np.float32)], axis=1
    ).astype(np.float32)
    return {
        "wbaseT": wbaseT, "negw1bT": negw1bT, "w2T": w2T, "id16": id16,
        "negbigI": negbigI, "neghalf_mat": neghalf_mat, "gb": gb,
    }


def kernel(features, w1, b1, g1, beta1, w2, b2, g2, beta2):
    from concourse.bass_utils import run_bass_kernel_spmd

    if "nc" not in _CACHE:
        _CACHE["nc"] = build(collectives=True)
    nc = _CACHE["nc"]

    x = np.ascontiguousarray(np.asarray(features, np.float32).reshape(B, C, N))
    shared = _prep_shared(w1, w2, g1, beta1, g2, beta2)
    in_maps = [{"x": x[b], **shared} for b in range(B)]
    res = run_bass_kernel_spmd(nc, in_maps, core_ids=list(range(B)))
    out = np.stack([res.results[b]["out"] for b in range(B)])
    return out.reshape(B, C, N, 1)



# revision 10
# speedup vs baseline: 1.1187x; 1.1187x over previous
"""DGCNN block (knn -> edge-conv -> BN/ReLU -> conv -> BN/ReLU) on 8 trn2
NeuronCores, data-parallel over the batch (one sample per core).

v3 restructuring vs v2:
  - h assembly moved from DVE tensor-adds to PE PSUM accumulation: gathers
    fetch raw x columns (f16) and conv1 is computed as 3-4 matmuls per
    (quarter, chunk, j) accumulating base + gathered terms in PSUM.  negY
    pre-products and the DVE adds disappear.
  - BN1 statistics via Activation-engine accumulators: the h PSUM->SBUF
    copy carries accum_out (sum h); a second Square pass carries sum h^2.
    DVE bn_stats for BN1 disappears; BN1 uses raw-sum math.
  - optional 4-pass KNN (slab computed from f16(xsq) only) -- SLAB_LO flag.
  - DVE now runs only the Max/MaxIndex scans (plus tiny BN chains), which
    is the hard floor of this algorithm on trn2.
"""
import sys

sys.path.insert(0, "/opt/trn_rl_repo")

import numpy as np

B, C, N = 8, 128, 4096
NT = N // 128          # 32 row tiles
NCHUNK = N // 512      # 8 column chunks
EPS = 1e-5
NEGBIG = -30000.0
SLAB_LO = True         # include the xsqlo slab pass (5-pass KNN)

_CACHE = {}


# --------------------------------------------------------------------------
# workarounds for this walrus build (small sem-wait encodings)
# --------------------------------------------------------------------------

def _patched_drain_and_barrier(self, tick_clock, wait_clock):
    from concourse.vector_clock import ScopedClock, VectorClock

    nc = self.nc
    gc = tick_clock.global_clock
    n = len(gc)
    for p in range(n):
        t = gc[p]
        if t > 0:
            vc = VectorClock([0] * n)
            vc.require_at_least(p, t)
            w = nc.sync.nop()
            wait_clock.add_sem_waits(w.ins, ScopedClock({None: vc}))
    nc.sync.drain()
    nc.all_engine_barrier()
    assert self.sems is not None
    popped = nc._tile_sem_poison_stack.pop()
    assert popped is self._sem_poison
    nc.clear_and_free_semaphores(list(self.sems.allocated().values()))
    nc.all_engine_barrier()


def _split_excess_waits(nc, cap=1):
    import concourse.mybir as mybir

    for f in nc.m.functions:
        for bb in f.blocks:
            il = bb.instructions
            k = 0
            while k < len(il):
                inst = il[k]
                si = inst.sync_info
                if si is None or not si.on_wait or len(si.on_wait) <= cap:
                    k += 1
                    continue
                waits = list(si.on_wait)
                keep, excess = waits[-cap:], waits[:-cap]
                pos = k
                for i0 in range(0, len(excess), cap):
                    chunk = excess[i0:i0 + cap]
                    nop = mybir.InstNoOp(
                        name=f"{inst.name}-wsplit{i0}", ins=[], outs=[]
                    )
                    nop.engine = inst.engine
                    nop.sync_info = mybir.SyncInfo(on_wait=chunk, on_update=[])
                    il.insert(pos, nop)
                    pos += 1
                    k += 1
                inst.sync_info = mybir.SyncInfo(
                    on_wait=keep, on_update=list(si.on_update or [])
                )
                k += 1


# --------------------------------------------------------------------------
# device program
# --------------------------------------------------------------------------

def build(collectives=True):
    import concourse.bass as bass
    import concourse.tile as tile
    import concourse.mybir as mybir
    from concourse.library_overlay import lower_extended_insts

    tile.TileContext._drain_and_barrier = _patched_drain_and_barrier

    f32 = mybir.dt.float32
    f16 = mybir.dt.float16
    u16 = mybir.dt.uint16
    Ident = mybir.ActivationFunctionType.Identity
    Square = mybir.ActivationFunctionType.Square

    nc = bass.Bass()

    x_d = nc.dram_tensor("x", [C, N], f32, kind="ExternalInput")
    wbase_d = nc.dram_tensor("wbaseT", [C, C], f16, kind="ExternalInput")
    wj0_d = nc.dram_tensor("wj0T", [C, C], f16, kind="ExternalInput")
    negw1b_d = nc.dram_tensor("negw1bT", [C, 3 * C], f16, kind="ExternalInput")
    w2t_d = nc.dram_tensor("w2T", [C, 3 * C], f16, kind="ExternalInput")
    id16_d = nc.dram_tensor("id16", [C, C], f16, kind="ExternalInput")
    negbig_d = nc.dram_tensor("negbigI", [C, C], f16, kind="ExternalInput")
    nhm_d = nc.dram_tensor("neghalf_mat", [C, C], f32, kind="ExternalInput")
    gb_d = nc.dram_tensor("gb", [C, 4], f32, kind="ExternalInput")  # g1,beta1,g2,beta2

    out_d = nc.dram_tensor("out", [C, N], f32, kind="ExternalOutput")

    with tile.TileContext(nc) as tc:
        with (
            tc.tile_pool(name="persist", bufs=1) as pp,
            tc.tile_pool(name="work", bufs=1) as wp,
            tc.tile_pool(name="small", bufs=1) as sp,
            tc.tile_pool(name="psum", bufs=2, space="PSUM") as psp,
            tc.tile_pool(name="dram", bufs=1, space="DRAM") as dp,
        ):
            # ---------- load ----------
            x32 = wp.tile([C, N], f32, tag="big32", bufs=3)
            for q in range(4):
                qs = slice(q * (N // 4), (q + 1) * (N // 4))
                nc.sync.dma_start(x32[:, qs], x_d[:, qs])
            wbase = pp.tile([C, C], f16)
            nc.sync.dma_start(wbase[:], wbase_d[:])
            wj0 = pp.tile([C, C], f16)
            nc.sync.dma_start(wj0[:], wj0_d[:])
            negw1b = pp.tile([C, 3 * C], f16)
            nc.sync.dma_start(negw1b[:], negw1b_d[:])
            w2t = pp.tile([C, 3 * C], f16)
            nc.sync.dma_start(w2t[:], w2t_d[:])
            id16 = pp.tile([C, C], f16)
            nc.sync.dma_start(id16[:], id16_d[:])
            negbig = pp.tile([C, C], f16)
            nc.sync.dma_start(negbig[:], negbig_d[:])
            nhm = pp.tile([C, C], f32)
            nc.sync.dma_start(nhm[:], nhm_d[:])
            nhm16 = pp.tile([C, C], f16)
            nc.scalar.copy(nhm16[:], nhm[:])
            gb = pp.tile([C, 4], f32)
            nc.sync.dma_start(gb[:], gb_d[:])

            # ---------- prep: hi/lo split, sq, slab ----------
            xhi = pp.tile([C, N], f16)
            xhi32 = wp.tile([C, N], f32, tag="big32", bufs=3)
            xlo = pp.tile([C, N], f16)
            xsq = wp.tile([C, N], f32, tag="big32", bufs=3)
            xsqhi = pp.tile([C, N], f16)
            if SLAB_LO:
                xsqhi32 = wp.tile([C, N], f32, tag="big32", bufs=3)
                xsqlo = pp.tile([C, N], f16)
            # per-quarter so tile 0's early chunks start before prep finishes
            for ph2 in range(4):
                hs = slice(ph2 * (N // 4), (ph2 + 1) * (N // 4))
                nc.scalar.copy(xhi[:, hs], x32[:, hs])
                nc.scalar.copy(xhi32[:, hs], xhi[:, hs])
                nc.vector.tensor_sub(xlo[:, hs], x32[:, hs], xhi32[:, hs])
                nc.vector.tensor_mul(xsq[:, hs], x32[:, hs], x32[:, hs])
                nc.scalar.copy(xsqhi[:, hs], xsq[:, hs])
                if SLAB_LO:
                    nc.scalar.copy(xsqhi32[:, hs], xsqhi[:, hs])
                    nc.vector.tensor_sub(xsqlo[:, hs], xsq[:, hs], xsqhi32[:, hs])

            # ---------- persistent state for the pipelined phase ----------
            idxall = pp.tile([C, NT * 8], u16)   # [p, r*8+k] idx of rank k+2
            iw = pp.tile([C, 4 * 512], u16)      # wrapped idx, per quarter:
            #                                      [:, qa*512 + (kk-1)*64 + rr*8 + q]
            h = [pp.tile([C, N], f16, name=f"h{j}", tag=f"h{j}") for j in range(3)]
            hsum = sp.tile([C, 24], f32, tag="hsum")
            hsqsum = sp.tile([C, 24], f32, tag="hsqsum")

            def emit_topk_tile(r):
                hi_t = xhi[:, r * 128:(r + 1) * 128]
                lo_t = xlo[:, r * 128:(r + 1) * 128]
                d = wp.tile([C, N], f32, tag="dtile", bufs=3)
                ckd = r // 4                       # chunk containing diagonal
                off = 128 * (r % 4)
                for qt in range(4):
                    ph = psp.tile([C, 1024], f32, tag="ph", bufs=3)
                    qs = slice(qt * 1024, (qt + 1) * 1024)
                    for c2 in range(2):
                        ck = qt * 2 + c2
                        sl = ph[:, c2 * 512:(c2 + 1) * 512]
                        rs = slice(ck * 512, (ck + 1) * 512)
                        nc.tensor.matmul(sl, nhm16[:], xsqhi[:, rs],
                                         start=True, stop=False)
                        if SLAB_LO:
                            nc.tensor.matmul(sl, nhm16[:], xsqlo[:, rs],
                                             start=False, stop=False)
                        nc.tensor.matmul(sl, hi_t, xhi[:, rs],
                                         start=False, stop=False)
                        nc.tensor.matmul(sl, hi_t, xlo[:, rs], start=False, stop=False)
                        if ck == ckd:
                            nc.tensor.matmul(sl, lo_t, xhi[:, rs], start=False, stop=False)
                            nc.tensor.matmul(
                                sl[:, off:off + 128], id16[:], negbig[:],
                                start=False, stop=True,
                            )
                        else:
                            nc.tensor.matmul(sl, lo_t, xhi[:, rs], start=False, stop=True)
                    nc.scalar.copy(d[:, qs], ph[:])
                v8 = sp.tile([C, 8], f32, tag="v8", bufs=2)
                nc.vector.max(v8[:], d[:])
                nc.vector.max_index(idxall[:, r * 8:(r + 1) * 8], v8[:], d[:])

            def emit_quarter_shuffle(qa, parallel=False):
                # idxall[:, qa*64:(qa+1)*64] -> wrapped iw[:, qa*512:(qa+1)*512]
                # parallel=True fans the small DMAs across 4 hwdge queues to
                # cut the serial latency (matters for the final quarter's tail)
                engs = ([nc.sync, nc.scalar, nc.gpsimd]
                        if parallel else [nc.sync])
                idxdram = dp.tile([8 * 128, 8], u16, name=f"idxd{qa}")
                nc.sync.dma_start(
                    idxdram[:].rearrange("(rr p) k -> p rr k", p=128),
                    idxall[:, qa * 64:(qa + 1) * 64].rearrange(
                        "c (rr k) -> c rr k", k=8
                    ),
                )
                iwq = sp.tile([16, 512], u16, tag=f"iwq{qa}")
                src = idxdram[:].rearrange(
                    "(rr q w) k -> w (rr q) k", rr=8, q=8, w=16
                )
                for k in range(8):
                    engs[k % len(engs)].dma_start(
                        iwq[:, k * 64:(k + 1) * 64],
                        src[:, :, k:k + 1].rearrange("w f a -> w (f a)"),
                    )
                for g in range(8):
                    engs[g % len(engs)].dma_start(
                        iw[g * 16:(g + 1) * 16, qa * 512:(qa + 1) * 512], iwq[:]
                    )

            gq = {}

            def emit_quarter_gathers(qa):
                gt = {}
                for kk in range(1, 9):
                    g = wp.tile([C, 1024], f16, tag=f"g{kk}", bufs=2)
                    nc.gpsimd.indirect_copy(
                        g[:], xhi[:],
                        iw[:, qa * 512 + (kk - 1) * 64:qa * 512 + kk * 64],
                        i_know_ap_gather_is_preferred=True,
                    )
                    gt[kk] = g
                gq[qa] = gt

            def emit_assemble_group(qa, gi):
                # one (c2, j) conv1 PSUM group; gi ordered j-major so that
                # group gi only needs gathers kk <= 3*(gi//2)+2 (emitted in
                # kk order), letting PE ride the gather burst without stalls
                gt = gq[qa]
                j, c2 = gi // 2, gi % 2
                cs = slice(qa * 1024 + c2 * 512, qa * 1024 + (c2 + 1) * 512)
                gs = slice(c2 * 512, (c2 + 1) * 512)
                ps = psp.tile([C, 512], f32, tag="ph0", bufs=2)
                if j == 0:
                    nc.tensor.matmul(ps[:], wj0[:], xhi[:, cs],
                                     start=True, stop=False)
                    nc.tensor.matmul(ps[:], negw1b[:, C:2 * C],
                                     gt[1][:, gs], start=False, stop=False)
                    nc.tensor.matmul(ps[:], negw1b[:, 2 * C:3 * C],
                                     gt[2][:, gs], start=False, stop=True)
                else:
                    nc.tensor.matmul(ps[:], wbase[:], xhi[:, cs],
                                     start=True, stop=False)
                    for t in range(3):
                        kk = 3 * j + t
                        nc.tensor.matmul(
                            ps[:], negw1b[:, t * C:(t + 1) * C],
                            gt[kk][:, gs], start=False, stop=(t == 2),
                        )
                slot = j * 8 + qa * 2 + c2
                nc.scalar.activation(
                    h[j][:, cs], ps[:], Ident, bias=0.0,
                    accum_out=hsum[:, slot:slot + 1],
                )
                sq = wp.tile([C, 512], f16, tag="hsq", bufs=2)
                nc.scalar.activation(
                    sq[:], h[j][:, cs], Square,
                    accum_out=hsqsum[:, slot:slot + 1],
                )

            # ---------- pipelined KNN + gather + assemble ----------
            # assembly group gi of quarter qa is emitted at tile 8*qa+12+gi,
            # well after the gather burst for qa (starting ~tile 8*qa+8)
            # has produced the gathers that group needs; leftovers run
            # post-loop, riding the final quarter's gather burst.
            emitted = set()
            for r in range(NT):
                emit_topk_tile(r)
                if r % 8 == 7 and r < NT - 1:
                    qa = r // 8
                    emit_quarter_shuffle(qa)
                    emit_quarter_gathers(qa)
                for gi in range(6):
                    if r - 12 - gi >= 0 and (r - 12 - gi) % 8 == 0:
                        qa = (r - 12 - gi) // 8
                        emit_assemble_group(qa, gi)
                        emitted.add((qa, gi))
            emit_quarter_shuffle(3, parallel=True)
            emit_quarter_gathers(3)
            for qa in range(4):
                for gi in range(6):
                    if (qa, gi) not in emitted:
                        emit_assemble_group(qa, gi)

            # ---------- BN1 reduce (raw sums) ----------
            pay = sp.tile([C, 2], f32, tag="pay")
            nc.vector.tensor_reduce(pay[:, 0:1], hsum[:],
                                    axis=mybir.AxisListType.X,
                                    op=mybir.AluOpType.add)
            nc.vector.tensor_reduce(pay[:, 1:2], hsqsum[:],
                                    axis=mybir.AxisListType.X,
                                    op=mybir.AluOpType.add)

            if collectives:
                cin = dp.tile([C, 2], f32)
                cout = dp.tile([C, 2], f32)
                nc.gpsimd.dma_start(cin[:], pay[:])
                nc.gpsimd.collective_compute(
                    "AllReduce", mybir.AluOpType.add,
                    replica_groups=[list(range(B))],
                    ins=[cin[:]], outs=[cout[:]],
                )
                red = sp.tile([C, 2], f32, tag="red")
                nc.gpsimd.dma_start(red[:], cout[:])
                scale_n = 1.0 / (B * 3 * N)
            else:
                red = pay
                scale_n = 1.0 / (3 * N)

            # sc1 = g1 * rsqrt(var_g + eps); bi1 = beta1 - mean_g * sc1
            mean_g = sp.tile([C, 1], f32, tag="t2")
            nc.vector.tensor_scalar_mul(mean_g[:], red[:, 0:1], scale_n)
            ex2 = sp.tile([C, 1], f32, tag="t3")
            nc.vector.tensor_scalar_mul(ex2[:], red[:, 1:2], scale_n)
            mg2 = sp.tile([C, 1], f32, tag="t4")
            nc.vector.tensor_mul(mg2[:], mean_g[:], mean_g[:])
            var_g = sp.tile([C, 1], f32, tag="t5")
            nc.vector.tensor_sub(var_g[:], ex2[:], mg2[:])
            veps = sp.tile([C, 1], f32, tag="t6b")
            nc.vector.tensor_scalar_add(veps[:], var_g[:], EPS)
            sd = sp.tile([C, 1], f32, tag="t6")
            nc.scalar.activation(
                sd[:], veps[:], mybir.ActivationFunctionType.Sqrt
            )
            rst = sp.tile([C, 1], f32, tag="t7")
            nc.vector.reciprocal(rst[:], sd[:])
            sc1 = sp.tile([C, 1], f32, tag="sc1")
            nc.vector.tensor_mul(sc1[:], gb[:, 0:1], rst[:])
            tmp1 = sp.tile([C, 1], f32, tag="t8")
            nc.vector.tensor_mul(tmp1[:], mean_g[:], sc1[:])
            bi1 = sp.tile([C, 1], f32, tag="bi1")
            nc.vector.tensor_sub(bi1[:], gb[:, 1:2], tmp1[:])

            # ---------- BN1 apply + conv2, pipelined per quarter ----------
            o2 = wp.tile([C, N], f32, tag="big32", bufs=3)
            stats2 = sp.tile([C, NCHUNK * 6], f32, tag="stats2")
            zero1 = sp.tile([C, 1], f32, tag="z1")
            nc.vector.memset(zero1[:], 0.0)
            for qa in range(4):
                qs = slice(qa * 1024, (qa + 1) * 1024)
                for j in range(3):
                    if (qa * 3 + j) % 2 == 0:
                        nc.scalar.activation(
                            h[j][:, qs], h[j][:, qs],
                            mybir.ActivationFunctionType.Relu,
                            bias=bi1[:], scale=sc1[:],
                        )
                    else:
                        # relu(h*sc1+bi1) = max(h*sc1+bi1, 0) on DVE
                        nc.vector.scalar_tensor_tensor(
                            h[j][:, qs], h[j][:, qs], sc1[:],
                            bi1[:].broadcast_to([C, 1024]),
                            op0=mybir.AluOpType.mult,
                            op1=mybir.AluOpType.add,
                        )
                        nc.vector.tensor_scalar_max(h[j][:, qs], h[j][:, qs], zero1[:])
                for c2 in range(2):
                    ck = qa * 2 + c2
                    ps = psp.tile([C, 512], f32, tag="ph0", bufs=2)
                    for j in range(3):
                        nc.tensor.matmul(
                            ps[:], w2t[:, j * C:(j + 1) * C],
                            h[j][:, ck * 512:(ck + 1) * 512],
                            start=(j == 0), stop=(j == 2),
                        )
                    nc.scalar.copy(o2[:, ck * 512:(ck + 1) * 512], ps[:])
                    nc.vector.bn_stats(
                        stats2[:, ck * 6:(ck + 1) * 6],
                        o2[:, ck * 512:(ck + 1) * 512],
                    )

            # ---------- BN2 ----------
            mv2 = sp.tile([C, 2], f32, tag="mv2")
            nc.vector.bn_aggr(mv2[:], stats2[:].rearrange("c (s k) -> c s k", k=6))
            pay2 = sp.tile([C, 2], f32, tag="pay2")
            nc.vector.tensor_copy(pay2[:, 0:1], mv2[:, 0:1])
            msq2 = sp.tile([C, 1], f32, tag="u1")
            nc.vector.tensor_mul(msq2[:], mv2[:, 0:1], mv2[:, 0:1])
            nc.vector.tensor_add(pay2[:, 1:2], mv2[:, 1:2], msq2[:])

            if collectives:
                cin2 = dp.tile([C, 2], f32)
                cout2 = dp.tile([C, 2], f32)
                nc.gpsimd.dma_start(cin2[:], pay2[:])
                nc.gpsimd.collective_compute(
                    "AllReduce", mybir.AluOpType.add,
                    replica_groups=[list(range(B))],
                    ins=[cin2[:]], outs=[cout2[:]],
                )
                red2 = sp.tile([C, 2], f32, tag="red2")
                nc.gpsimd.dma_start(red2[:], cout2[:])
                scale2 = 1.0 / B
            else:
                red2 = pay2
                scale2 = 1.0

            mean2 = sp.tile([C, 1], f32, tag="u2")
            nc.vector.tensor_scalar_mul(mean2[:], red2[:, 0:1], scale2)
            ex22 = sp.tile([C, 1], f32, tag="u3")
            nc.vector.tensor_scalar_mul(ex22[:], red2[:, 1:2], scale2)
            mg22 = sp.tile([C, 1], f32, tag="u4")
            nc.vector.tensor_mul(mg22[:], mean2[:], mean2[:])
            var2 = sp.tile([C, 1], f32, tag="u5")
            nc.vector.tensor_sub(var2[:], ex22[:], mg22[:])
            veps2 = sp.tile([C, 1], f32, tag="u6b")
            nc.vector.tensor_scalar_add(veps2[:], var2[:], EPS)
            sd2 = sp.tile([C, 1], f32, tag="u6")
            nc.scalar.activation(
                sd2[:], veps2[:], mybir.ActivationFunctionType.Sqrt
            )
            rst2 = sp.tile([C, 1], f32, tag="u7")
            nc.vector.reciprocal(rst2[:], sd2[:])
            sc2 = sp.tile([C, 1], f32, tag="sc2")
            nc.vector.tensor_mul(sc2[:], gb[:, 2:3], rst2[:])
            tmp2 = sp.tile([C, 1], f32, tag="u8")
            nc.vector.tensor_mul(tmp2[:], mean2[:], sc2[:])
            bi2 = sp.tile([C, 1], f32, tag="bi2")
            nc.vector.tensor_sub(bi2[:], gb[:, 3:4], tmp2[:])

            for hh in range(4):
                hs = slice(hh * 1024, (hh + 1) * 1024)
                nc.scalar.activation(
                    o2[:, hs], o2[:, hs], mybir.ActivationFunctionType.Relu,
                    bias=bi2[:], scale=sc2[:],
                )
                nc.sync.dma_start(out_d[:, hs], o2[:, hs])

    lower_extended_insts(nc)
    _split_excess_waits(nc)
    return nc


# --------------------------------------------------------------------------
# host wrapper
# --------------------------------------------------------------------------

def _prep_shared(w1, w2, g1, beta1, g2, beta2):
    w1 = np.asarray(w1, np.float32)
    w2 = np.asarray(w2, np.float32)
    W1A, W1B = w1[:, :C, :], w1[:, C:, :]
    wbaseT = (W1A + W1B).sum(2).T.astype(np.float16).copy()
    wj0T = ((W1A + W1B).sum(2) - W1B[:, :, 0]).T.astype(np.float16).copy()
    negw1bT = np.concatenate(
        [(-W1B[:, :, t]).T for t in range(3)], axis=1
    ).astype(np.float16)
    w2T = np.concatenate([w2[:, :, j].T for j in range(3)], axis=1).astype(np.float16)
    id16 = np.eye(C, dtype=np.float16)
    negbigI = (NEGBIG * np.eye(C)).astype(np.float16)
    neghalf_mat = np.full((C, C), -0.5, np.float32)
    gb = np.stack(
        [np.asarray(g1, np.float32), np.asarray(beta1, np.float32),
         np.asarray(g2, np.float32), np.asarray(beta2, np.float32)], axis=1
    ).astype(np.float32)
    return {
        "wbaseT": wbaseT, "wj0T": wj0T, "negw1bT": negw1bT, "w2T": w2T,
        "id16": id16, "negbigI": negbigI, "neghalf_mat": neghalf_mat, "gb": gb,
    }


def kernel(features, w1, b1, g1, beta1, w2, b2, g2, beta2):
    from concourse.bass_utils import run_bass_kernel_spmd

    if "nc" not in _CACHE:
        _CACHE["nc"] = build(collectives=True)
    nc = _CACHE["nc"]

    x = np.ascontiguousarray(np.asarray(features, np.float32).reshape(B, C, N))
    shared = _prep_shared(w1, w2, g1, beta1, g2, beta2)
    in_maps = [{"x": x[b], **shared} for b in range(B)]
    res = run_bass_kernel_spmd(nc, in_maps, core_ids=list(range(B)))
    out = np.stack([res.results[b]["out"] for b in range(B)])
    return out.reshape(B, C, N, 1)
